# revision 25
# baseline (speedup 1.0000x reference)
"""Causal self-attention on 8 trn2 NeuronCores.

Sharding (batch+head hint): core c handles batch b = c//2 (data parallel) and
head-group g = c%2 (8 of 16 heads; tensor-parallel slice of w_qkv columns /
w_out rows). Each core computes a full-batch-slice partial of the output
projection over its 512 head dims; the two partials per batch are summed on
gather (the "all-reduce after out_proj").

Kernel dataflow per core (S=2048 tokens, D=1024, 8 heads x 64):
  - x arrives pre-transposed from the host (xT [D, S] as [128, ki*S]), so
    phase 1 is pure matmuls: qT/kT = W^T x^T with W chunks stationary; v is
    computed directly in natural [token, dim] orientation with xT chunks
    stationary (no PE transposes anywhere).
  - attention per (head, 512-query block): exact-causal flash in transposed
    orientation, all bf16 on the PE. Key chunks processed in PAIRS packed
    contiguously into one 2-bank PSUM tile so a single exp instruction covers
    both (halves act-engine overhead). Scores run LA=3 pair-batches ahead of
    the av matmuls so every LDWEIGHTS wait is satisfied early and weight loads
    hide under the previous matmul's stream. The ones-column appended to v
    accumulates the softmax denominator; normalize = reciprocal + gpsimd
    partition_broadcast + vector mul into oT.
  - out-proj chains (oT chunks stationary, W_out moving) and next block's qkv
    chains are kept in a fill queue and interleaved into the attention
    emission, so the PE streams continuously while the act engine paces exp.
"""
from collections import deque

import numpy as np

B = 4
S = 2048
D = 1024
HG = 8           # heads per core
DH = 64
NCORES = 8
NB = S // 512    # 512-token blocks
KC = D // 128    # contraction chunks over D

_CACHE = {}


def _build_nc():
    import concourse.bass as bass  # noqa
    import concourse.mybir as mybir
    import concourse.tile as tile
    from concourse import bacc

    F32 = mybir.dt.float32
    BF = mybir.dt.bfloat16
    Exp = mybir.ActivationFunctionType.Exp

    nc = bacc.Bacc("TRN2", target_bir_lowering=False, debug=False,
                   enable_asserts=False, num_devices=NCORES)
    xt_d = nc.dram_tensor("xt", [128, KC * S], BF, kind="ExternalInput")
    wqkv_d = nc.dram_tensor("wqkv", [128, KC * 1536], BF, kind="ExternalInput")
    wout_d = nc.dram_tensor("wout", [128, 4 * D], BF, kind="ExternalInput")
    masks_d = nc.dram_tensor("masks", [128, 128], F32, kind="ExternalInput")
    # bf16 partials: host sums the two head-group partials in fp32
    out_d = nc.dram_tensor("out", [S, D], BF, kind="ExternalOutput")

    with tile.TileContext(nc) as tc:
        with tc.tile_pool(name="persist", bufs=1) as persist, \
             tc.tile_pool(name="probs", bufs=6) as pr_pool, \
             tc.tile_pool(name="recip", bufs=2) as rc_pool, \
             tc.tile_pool(name="rbc", bufs=2) as rb_pool, \
             tc.tile_pool(name="ostage", bufs=6) as ost_pool, \
             tc.tile_pool(name="tmpa", bufs=8) as tmpa_pool, \
             tc.tile_pool(name="ps_sc", bufs=2, space="PSUM") as ps_sc, \
             tc.tile_pool(name="ps_fill", bufs=2, space="PSUM") as ps_fill, \
             tc.tile_pool(name="ps_av", bufs=2, space="PSUM") as ps_av:
            qT = persist.tile([128, 4 * S], BF)
            kT = persist.tile([128, 4 * S], BF)
            v1 = persist.tile([128, HG * 16 * 65], BF)
            oT = persist.tile([128, 4 * S], BF)
            xT_sb = persist.tile([128, KC * S], BF)
            w_sb = persist.tile([128, KC * 1536], BF)
            wout_sb = persist.tile([128, 4 * D], BF)
            tri = persist.tile([128, 128], F32)
            dummy = persist.tile([1, 32], F32)

            # ones columns of every [*, 65] v slot (softmax denominator trick)
            ones128 = persist.tile([128, 128], F32)
            nc.vector.memset(ones128[:], 1.0)
            nc.scalar.copy(
                v1[:].rearrange("p (s u) -> p s u", u=65)[:, :, 64:65],
                ones128[:].rearrange("p (c u) -> p c u", u=1))
            # warm the act engine's Exp table while DMAs stream in
            nc.vector.memset(dummy[:], 0.0)
            nc.scalar.activation(dummy[:], dummy[:], Exp)
            nc.sync.dma_start(tri[:], masks_d.ap())
            # warm-up matmuls: ramp the PE clock to full p-state while the
            # first input DMAs are still landing (results never read)
            ones_bf = persist.tile([128, 128], BF)
            nc.vector.memset(ones_bf[:], 1.0)
            warm = ps_fill.tile([128, 512], F32, tag="pf", name="warm")
            for _ in range(200):
                nc.tensor.matmul(warm[:, 0:128], ones_bf[:], ones_bf[:],
                                 start=True, stop=True)

            # --- input DMAs: few large transfers, one per queue; the 16
            # priority DMAs cover exactly what the first qkv chains need ---
            for ki in range(KC):
                nc.sync.dma_start(
                    xT_sb[:, ki * S: ki * S + 512],
                    xt_d[:, ki * S: ki * S + 512])
            for ki in range(KC):
                nc.sync.dma_start(
                    w_sb[:, ki * 1536: ki * 1536 + 256],
                    wqkv_d[:, ki * 1536: ki * 1536 + 256])
            # rest of w and x, one DMA per ki
            for ki in range(KC):
                nc.sync.dma_start(
                    w_sb[:, ki * 1536 + 256: (ki + 1) * 1536],
                    wqkv_d[:, ki * 1536 + 256: (ki + 1) * 1536])
            for ki in range(KC):
                nc.sync.dma_start(
                    xT_sb[:, ki * S + 512: (ki + 1) * S],
                    xt_d[:, ki * S + 512: (ki + 1) * S])
            for k in range(2):
                nc.sync.dma_start(
                    wout_sb[:, k * 2 * D:(k + 1) * 2 * D],
                    wout_d[:, k * 2 * D:(k + 1) * 2 * D])

            # --- fill-work chains (each thunk emits one PE chain + copy) ---
            def qk_chain(tb, m):
                acc = ps_fill.tile([128, 512], F32, tag="pf",
                                   name=f"qk_{tb}_{m}")
                for ki in range(KC):
                    nc.tensor.matmul(
                        acc[:],
                        w_sb[:, ki * 1536 + m * 128: ki * 1536 + (m + 1) * 128],
                        xT_sb[:, ki * S + tb * 512: ki * S + (tb + 1) * 512],
                        start=(ki == 0), stop=(ki == KC - 1))
                dst = qT if m < 4 else kT
                nc.vector.tensor_copy(
                    dst[:, (m % 4) * S + tb * 512:(m % 4) * S + tb * 512 + 512],
                    acc[:])

            def vnat_chain(tb, t):
                acc = ps_fill.tile([128, 512], F32, tag="pf",
                                   name=f"vn_{tb}_{t}")
                for ki in range(KC):
                    nc.tensor.matmul(
                        acc[:],
                        xT_sb[:, ki * S + tb * 512 + t * 128:
                              ki * S + tb * 512 + (t + 1) * 128],
                        w_sb[:, ki * 1536 + 1024: ki * 1536 + 1536],
                        start=(ki == 0), stop=(ki == KC - 1))
                sck = tb * 4 + t
                nc.vector.tensor_copy(
                    v1[:].rearrange("p (h c u) -> p h c u", h=HG, u=65)
                    [:, :, sck, 0:64],
                    acc[:].rearrange("p (h d) -> p h d", h=HG))

            def _out_dma(m, half, ost, split=1):
                w = 512 // split
                for qh in range(split):
                    nc.sync.dma_start(
                        out_d[m * 128:(m + 1) * 128,
                              half * 512 + qh * w: half * 512 + (qh + 1) * w],
                        ost[:, qh * w:(qh + 1) * w])

            def proj_chain(m, half):
                pso = ps_fill.tile([128, 512], F32, tag="pf",
                                   name=f"po_{m}_{half}")
                for k in range(4):
                    nc.tensor.matmul(
                        pso[:],
                        oT[:, k * S + m * 128: k * S + m * 128 + 128],
                        wout_sb[:, k * D + half * 512: k * D + half * 512 + 512],
                        start=(k == 0), stop=(k == 3))
                ost = ost_pool.tile([128, 512], BF, tag="ost",
                                    name=f"ost_{m}_{half}")
                nc.vector.tensor_copy(ost[:], pso[:])
                _out_dma(m, half, ost)

            # last block's proj is split: k=0..2 runs as late fills while the
            # final heads' attention drains; only the k=3 matmul + add remain
            # after the last normalize, keeping the PE busy through the tail
            tmpa = {}

            def projA_chain(m, half):
                pso = ps_fill.tile([128, 512], F32, tag="pf",
                                   name=f"pa_{m}_{half}")
                for k in range(3):
                    nc.tensor.matmul(
                        pso[:],
                        oT[:, k * S + m * 128: k * S + m * 128 + 128],
                        wout_sb[:, k * D + half * 512: k * D + half * 512 + 512],
                        start=(k == 0), stop=(k == 2))
                t = tmpa_pool.tile([128, 512], F32, tag="tmpa",
                                   name=f"tmpa_{m}_{half}")
                # scalar engine: keeps the DVE free for the final normalizes
                nc.scalar.copy(t[:], pso[:])
                tmpa[(m, half)] = t

            def projB_tail(m, half):
                pso = ps_fill.tile([128, 512], F32, tag="pf",
                                   name=f"pb_{m}_{half}")
                nc.tensor.matmul(
                    pso[:],
                    oT[:, 3 * S + m * 128: 3 * S + m * 128 + 128],
                    wout_sb[:, 3 * D + half * 512: 3 * D + half * 512 + 512],
                    start=True, stop=True)
                ost = ost_pool.tile([128, 512], BF, tag="ost",
                                    name=f"ostb_{m}_{half}")
                nc.vector.tensor_add(ost[:], pso[:], tmpa.pop((m, half))[:])
                _out_dma(m, half, ost, split=4)

            def qkv_thunks(tb):
                tks = [(lambda tb=tb, m=m: qk_chain(tb, m)) for m in range(8)]
                tks += [(lambda tb=tb, t=t: vnat_chain(tb, t)) for t in range(4)]
                return tks

            def proj_thunks(tb):
                return [(lambda m=m, h=h: proj_chain(m, h))
                        for m in range(4 * tb, 4 * tb + 4) for h in range(2)]

            fill_q = deque()

            # --- attention: pair-batched, software-pipelined, fill-filled ---
            def emit_sc_pair(h, j, tb):
                r, po = h // 2, 64 * (h % 2)
                sA, sB = 2 * j, 2 * j + 1
                loA = max(128 * sA - 512 * tb, 0)
                loB = max(128 * sB - 512 * tb, 0)
                nA, nB = 512 - loA, 512 - loB
                # chunk A at psum cols [0:nA] (bank 0), chunk B at [512:512+nB]
                # (bank 1) so each start=True matmul owns a whole bank
                ps = ps_sc.tile([128, 1024], F32, tag="ps",
                                name=f"sc_{tb}_{h}_{j}")
                nc.tensor.matmul(
                    ps[:, 0:nA],
                    kT[po:po + 64, r * S + sA * 128: r * S + sA * 128 + 128],
                    qT[po:po + 64, r * S + 512 * tb + loA: r * S + 512 * (tb + 1)],
                    start=True, stop=True)
                nc.tensor.matmul(
                    ps[:, 512:512 + nB],
                    kT[po:po + 64, r * S + sB * 128: r * S + sB * 128 + 128],
                    qT[po:po + 64, r * S + 512 * tb + loB: r * S + 512 * (tb + 1)],
                    start=True, stop=True)
                if sA >= 4 * tb:
                    nc.vector.tensor_add(ps[:, 0:128], ps[:, 0:128], tri[:])
                if sB >= 4 * tb:
                    nc.vector.tensor_add(
                        ps[:, 512:512 + 128], ps[:, 512:512 + 128], tri[:])
                pr = pr_pool.tile([128, 1024], BF, tag="pr",
                                  name=f"pr_{tb}_{h}_{j}")
                if nA == 512:
                    # valid region is contiguous [0 : 512+nB] — one exp
                    nc.scalar.activation(
                        pr[:, 0:512 + nB], ps[:, 0:512 + nB], Exp)
                else:
                    nc.scalar.activation(pr[:, 0:nA], ps[:, 0:nA], Exp)
                    nc.scalar.activation(
                        pr[:, 512:512 + nB], ps[:, 512:512 + nB], Exp)
                return pr

            av_acc = {}

            def emit_av_pair(h, j, tb, pr):
                r, po = h // 2, 64 * (h % 2)
                sA, sB = 2 * j, 2 * j + 1
                loA = max(128 * sA - 512 * tb, 0)
                loB = max(128 * sB - 512 * tb, 0)
                nA, nB = 512 - loA, 512 - loB
                if j == 0:
                    av_acc[h] = ps_av.tile([128, 512], F32, tag="pa",
                                           name=f"av_{tb}_{h}")
                acc = av_acc[h]
                nc.tensor.matmul(
                    acc[0:65, loA:512],
                    v1[:, (h * 16 + sA) * 65:(h * 16 + sA) * 65 + 65],
                    pr[:, 0:nA],
                    start=(j == 0), stop=False)
                last = (j == 2 * tb + 1)
                nc.tensor.matmul(
                    acc[0:65, loB:512],
                    v1[:, (h * 16 + sB) * 65:(h * 16 + sB) * 65 + 65],
                    pr[:, 512:512 + nB],
                    start=False, stop=last)
                if last:
                    acc = av_acc.pop(h)
                    den = rc_pool.tile([1, 512], F32, tag="den")
                    nc.vector.tensor_copy(den[:], acc[64:65, :])
                    rc = rc_pool.tile([1, 512], F32, tag="rc")
                    nc.vector.reciprocal_approx_fast(rc[:], den[:])
                    rb = rb_pool.tile([64, 512], F32, tag="rb")
                    nc.gpsimd.partition_broadcast(rb[:], rc[:])
                    nc.vector.tensor_mul(
                        oT[po:po + 64, r * S + 512 * tb: r * S + 512 * tb + 512],
                        acc[0:64, :], rb[:])

            LA = 4

            def attention_block(tb, late_fills=None):
                batches = [(h, j) for h in range(HG) for j in range(2 * tb + 2)]
                nbatch = len(batches)
                late_start = 54
                fills_per = len(fill_q) / nbatch
                # spread late fills so ~3 chains remain to drain after the
                # final av pairs, covering the last normalize latency
                late_per = (len(late_fills) / (nbatch - late_start + 4)
                            if late_fills else 0.0)
                fill_credit = 0.0
                late_credit = 0.0
                pending = deque()
                for idx, (h, j) in enumerate(batches):
                    pr = emit_sc_pair(h, j, tb)
                    pending.append((h, j, pr))
                    if idx >= LA:
                        h0, j0, pr0 = pending.popleft()
                        emit_av_pair(h0, j0, tb, pr0)
                    fill_credit += fills_per
                    while fill_credit >= 1.0 and fill_q:
                        fill_q.popleft()()
                        fill_credit -= 1.0
                    if late_fills and idx >= late_start:
                        late_credit += late_per
                        while late_credit >= 1.0 and late_fills:
                            late_fills.popleft()()
                            late_credit -= 1.0
                while pending:
                    h0, j0, pr0 = pending.popleft()
                    emit_av_pair(h0, j0, tb, pr0)
                while fill_q:
                    fill_q.popleft()()
                if late_fills:
                    while late_fills:
                        late_fills.popleft()()

            # block 0 qkv inline, then attention blocks with fills
            for t in qkv_thunks(0):
                t()
            for tb in range(NB):
                if tb + 1 < NB:
                    fill_q.extend(qkv_thunks(tb + 1))
                if tb >= 1:
                    fill_q.extend(proj_thunks(tb - 1))
                late = None
                if tb == NB - 1:
                    late = deque(
                        (lambda m=m, h=h: projA_chain(m, h))
                        for m in range(12, 16) for h in range(2))
                attention_block(tb, late)
            for m in range(12, 16):
                for h in range(2):
                    projB_tail(m, h)
    nc.compile()
    return nc


def _make_masks():
    # tri[p, c] = 0 if c >= p else -1e10 (lower-triangle additive mask applied
    # to the first 128 valid columns of each diagonal key chunk)
    p = np.arange(128)[:, None]
    c = np.arange(128)[None, :]
    return np.where(c >= p, 0.0, -1e10).astype(np.float32)


def _make_in_maps(x, w_qkv, w_out):
    import ml_dtypes
    bf = ml_dtypes.bfloat16
    masks = _make_masks()
    scale = np.float32(DH ** -0.5)
    in_maps = []
    for c in range(NCORES):
        g = c % 2
        wq = w_qkv[:, g * 512:(g + 1) * 512] * scale
        wk = w_qkv[:, D + g * 512: D + (g + 1) * 512]
        wv = w_qkv[:, 2 * D + g * 512: 2 * D + (g + 1) * 512]
        w = np.concatenate([wq, wk, wv], axis=1)  # [D, 1536]
        # [128, ki*1536]: partition p, chunk ki -> w[ki*128 + p, :]
        w = w.reshape(KC, 128, 1536).transpose(1, 0, 2).reshape(128, -1)
        # xT [D, S] -> [128, ki*S]
        xt = np.ascontiguousarray(x[c // 2].T)  # [D, S]
        xt = xt.reshape(KC, 128, S).transpose(1, 0, 2).reshape(128, -1)
        # wout slice [512, D] -> [128, k*D]
        wo = w_out[g * 512:(g + 1) * 512, :]
        wo = wo.reshape(4, 128, D).transpose(1, 0, 2).reshape(128, -1)
        in_maps.append({
            "xt": np.ascontiguousarray(xt).astype(bf),
            "wqkv": np.ascontiguousarray(w).astype(bf),
            "wout": np.ascontiguousarray(wo).astype(bf),
            "masks": masks,
        })
    return in_maps


def kernel(x, w_qkv, w_out):
    from concourse.bass_utils import run_bass_kernel_spmd

    x = np.asarray(x, dtype=np.float32)
    w_qkv = np.asarray(w_qkv, dtype=np.float32)
    w_out = np.asarray(w_out, dtype=np.float32)
    assert x.shape == (B, S, D) and w_qkv.shape == (D, 3 * D) and w_out.shape == (D, D)

    if "nc" not in _CACHE:
        _CACHE["nc"] = _build_nc()
    nc = _CACHE["nc"]

    in_maps = _make_in_maps(x, w_qkv, w_out)
    res = run_bass_kernel_spmd(nc, in_maps, core_ids=list(range(NCORES)),
                               trace=False)
    out = np.empty((B, S, D), dtype=np.float32)
    for b in range(B):
        out[b] = (np.asarray(res.results[2 * b]["out"], dtype=np.float32)
                  + np.asarray(res.results[2 * b + 1]["out"], dtype=np.float32))
    return out


# revision 26
# speedup vs baseline: 1.0226x; 1.0226x over previous
"""Causal self-attention on 8 trn2 NeuronCores.

Sharding (batch+head hint): core c handles batch b = c//2 (data parallel) and
head-group g = c%2 (8 of 16 heads; tensor-parallel slice of w_qkv columns /
w_out rows). Each core computes a full-batch-slice partial of the output
projection over its 512 head dims; the two partials per batch are summed on
gather (the "all-reduce after out_proj").

Kernel dataflow per core (S=2048 tokens, D=1024, 8 heads x 64):
  - x arrives pre-transposed from the host (xT [D, S] as [128, ki*S]), so
    phase 1 is pure matmuls: qT/kT = W^T x^T with W chunks stationary; v is
    computed directly in natural [token, dim] orientation with xT chunks
    stationary (no PE transposes anywhere).
  - attention per (head, 512-query block): exact-causal flash in transposed
    orientation, all bf16 on the PE. Key chunks processed in PAIRS packed
    contiguously into one 2-bank PSUM tile so a single exp instruction covers
    both (halves act-engine overhead). Scores run LA=3 pair-batches ahead of
    the av matmuls so every LDWEIGHTS wait is satisfied early and weight loads
    hide under the previous matmul's stream. The ones-column appended to v
    accumulates the softmax denominator; normalize = reciprocal + gpsimd
    partition_broadcast + vector mul into oT.
  - out-proj chains (oT chunks stationary, W_out moving) and next block's qkv
    chains are kept in a fill queue and interleaved into the attention
    emission, so the PE streams continuously while the act engine paces exp.
"""
from collections import deque

import numpy as np

B = 4
S = 2048
D = 1024
HG = 8           # heads per core
DH = 64
NCORES = 8
NB = S // 512    # 512-token blocks
KC = D // 128    # contraction chunks over D

_CACHE = {}


def _build_nc():
    import concourse.bass as bass  # noqa
    import concourse.mybir as mybir
    import concourse.tile as tile
    from concourse import bacc

    F32 = mybir.dt.float32
    BF = mybir.dt.bfloat16
    Exp = mybir.ActivationFunctionType.Exp

    nc = bacc.Bacc("TRN2", target_bir_lowering=False, debug=False,
                   enable_asserts=False, num_devices=NCORES)
    xt_d = nc.dram_tensor("xt", [128, KC * S], BF, kind="ExternalInput")
    wqkv_d = nc.dram_tensor("wqkv", [128, KC * 1536], BF, kind="ExternalInput")
    wout_d = nc.dram_tensor("wout", [128, 4 * D], BF, kind="ExternalInput")
    masks_d = nc.dram_tensor("masks", [128, 128], F32, kind="ExternalInput")
    # bf16 partials: host sums the two head-group partials in fp32
    out_d = nc.dram_tensor("out", [S, D], BF, kind="ExternalOutput")

    with tile.TileContext(nc) as tc:
        with tc.tile_pool(name="persist", bufs=1) as persist, \
             tc.tile_pool(name="probs", bufs=6) as pr_pool, \
             tc.tile_pool(name="recip", bufs=2) as rc_pool, \
             tc.tile_pool(name="rbc", bufs=2) as rb_pool, \
             tc.tile_pool(name="ostage", bufs=4) as ost_pool, \
             tc.tile_pool(name="tmpa", bufs=8) as tmpa_pool, \
             tc.tile_pool(name="ps_sc", bufs=2, space="PSUM") as ps_sc, \
             tc.tile_pool(name="ps_fill", bufs=2, space="PSUM") as ps_fill, \
             tc.tile_pool(name="ps_av", bufs=2, space="PSUM") as ps_av:
            qT = persist.tile([128, 4 * S], BF)
            kT = persist.tile([128, 4 * S], BF)
            v1 = persist.tile([128, HG * 16 * 65], BF)
            oT = persist.tile([128, 4 * S], BF)
            xT_sb = persist.tile([128, KC * S], BF)
            w_sb = persist.tile([128, KC * 1536], BF)
            wout_sb = persist.tile([128, 4 * D], BF)
            tri = persist.tile([128, 128], F32)
            dummy = persist.tile([1, 32], F32)

            # ones columns of every [*, 65] v slot (softmax denominator trick)
            ones128 = persist.tile([128, 128], F32)
            nc.vector.memset(ones128[:], 1.0)
            nc.scalar.copy(
                v1[:].rearrange("p (s u) -> p s u", u=65)[:, :, 64:65],
                ones128[:].rearrange("p (c u) -> p c u", u=1))
            # warm the act engine's Exp table while DMAs stream in
            nc.vector.memset(dummy[:], 0.0)
            nc.scalar.activation(dummy[:], dummy[:], Exp)
            nc.sync.dma_start(tri[:], masks_d.ap())
            # warm-up matmuls: ramp the PE clock to full p-state while the
            # first input DMAs are still landing (results never read)
            ones_bf = persist.tile([128, 128], BF)
            nc.vector.memset(ones_bf[:], 1.0)
            warm = ps_fill.tile([128, 512], F32, tag="pf", name="warm")
            for _ in range(110):
                nc.tensor.matmul(warm[:, 0:128], ones_bf[:], ones_bf[:],
                                 start=True, stop=True)

            # --- input DMAs: few large transfers, one per queue; the 16
            # priority DMAs cover exactly what the first qkv chains need ---
            for ki in range(KC):
                nc.sync.dma_start(
                    xT_sb[:, ki * S: ki * S + 512],
                    xt_d[:, ki * S: ki * S + 512])
            for ki in range(KC):
                nc.sync.dma_start(
                    w_sb[:, ki * 1536: ki * 1536 + 256],
                    wqkv_d[:, ki * 1536: ki * 1536 + 256])
            # rest of w and x, one DMA per ki
            for ki in range(KC):
                nc.sync.dma_start(
                    w_sb[:, ki * 1536 + 256: (ki + 1) * 1536],
                    wqkv_d[:, ki * 1536 + 256: (ki + 1) * 1536])
            for ki in range(KC):
                nc.sync.dma_start(
                    xT_sb[:, ki * S + 512: (ki + 1) * S],
                    xt_d[:, ki * S + 512: (ki + 1) * S])
            for k in range(2):
                nc.sync.dma_start(
                    wout_sb[:, k * 2 * D:(k + 1) * 2 * D],
                    wout_d[:, k * 2 * D:(k + 1) * 2 * D])

            # --- fill-work chains (each thunk emits one PE chain + copy) ---
            def qk_chain(tb, m):
                acc = ps_fill.tile([128, 512], F32, tag="pf",
                                   name=f"qk_{tb}_{m}")
                for ki in range(KC):
                    nc.tensor.matmul(
                        acc[:],
                        w_sb[:, ki * 1536 + m * 128: ki * 1536 + (m + 1) * 128],
                        xT_sb[:, ki * S + tb * 512: ki * S + (tb + 1) * 512],
                        start=(ki == 0), stop=(ki == KC - 1))
                dst = qT if m < 4 else kT
                nc.vector.tensor_copy(
                    dst[:, (m % 4) * S + tb * 512:(m % 4) * S + tb * 512 + 512],
                    acc[:])

            def vnat_chain(tb, t):
                acc = ps_fill.tile([128, 512], F32, tag="pf",
                                   name=f"vn_{tb}_{t}")
                for ki in range(KC):
                    nc.tensor.matmul(
                        acc[:],
                        xT_sb[:, ki * S + tb * 512 + t * 128:
                              ki * S + tb * 512 + (t + 1) * 128],
                        w_sb[:, ki * 1536 + 1024: ki * 1536 + 1536],
                        start=(ki == 0), stop=(ki == KC - 1))
                sck = tb * 4 + t
                nc.vector.tensor_copy(
                    v1[:].rearrange("p (h c u) -> p h c u", h=HG, u=65)
                    [:, :, sck, 0:64],
                    acc[:].rearrange("p (h d) -> p h d", h=HG))

            def _out_dma(m, half, ost, split=1):
                w = 512 // split
                for qh in range(split):
                    nc.sync.dma_start(
                        out_d[m * 128:(m + 1) * 128,
                              half * 512 + qh * w: half * 512 + (qh + 1) * w],
                        ost[:, qh * w:(qh + 1) * w])

            def proj_chain(m, half):
                pso = ps_fill.tile([128, 512], F32, tag="pf",
                                   name=f"po_{m}_{half}")
                for k in range(4):
                    nc.tensor.matmul(
                        pso[:],
                        oT[:, k * S + m * 128: k * S + m * 128 + 128],
                        wout_sb[:, k * D + half * 512: k * D + half * 512 + 512],
                        start=(k == 0), stop=(k == 3))
                ost = ost_pool.tile([128, 512], BF, tag="ost",
                                    name=f"ost_{m}_{half}")
                nc.vector.tensor_copy(ost[:], pso[:])
                _out_dma(m, half, ost)

            # last block's proj is split: k=0..2 runs as late fills while the
            # final heads' attention drains; only the k=3 matmul + add remain
            # after the last normalize, keeping the PE busy through the tail
            tmpa = {}

            def projA_chain(m, half):
                pso = ps_fill.tile([128, 512], F32, tag="pf",
                                   name=f"pa_{m}_{half}")
                for k in range(3):
                    nc.tensor.matmul(
                        pso[:],
                        oT[:, k * S + m * 128: k * S + m * 128 + 128],
                        wout_sb[:, k * D + half * 512: k * D + half * 512 + 512],
                        start=(k == 0), stop=(k == 2))
                t = tmpa_pool.tile([128, 512], F32, tag="tmpa",
                                   name=f"tmpa_{m}_{half}")
                # scalar engine: keeps the DVE free for the final normalizes
                nc.scalar.copy(t[:], pso[:])
                tmpa[(m, half)] = t

            def projB_tail(m, half):
                pso = ps_fill.tile([128, 512], F32, tag="pf",
                                   name=f"pb_{m}_{half}")
                nc.tensor.matmul(
                    pso[:],
                    oT[:, 3 * S + m * 128: 3 * S + m * 128 + 128],
                    wout_sb[:, 3 * D + half * 512: 3 * D + half * 512 + 512],
                    start=True, stop=True)
                ost = ost_pool.tile([128, 512], BF, tag="ost",
                                    name=f"ostb_{m}_{half}")
                nc.vector.tensor_add(ost[:], pso[:], tmpa.pop((m, half))[:])
                _out_dma(m, half, ost, split=2)

            def qkv_thunks(tb):
                tks = [(lambda tb=tb, m=m: qk_chain(tb, m)) for m in range(8)]
                tks += [(lambda tb=tb, t=t: vnat_chain(tb, t)) for t in range(4)]
                return tks

            def proj_thunks(tb):
                return [(lambda m=m, h=h: proj_chain(m, h))
                        for m in range(4 * tb, 4 * tb + 4) for h in range(2)]

            fill_q = deque()

            # --- attention: pair-batched, software-pipelined, fill-filled ---
            def emit_sc_pair(h, j, tb):
                r, po = h // 2, 64 * (h % 2)
                sA, sB = 2 * j, 2 * j + 1
                loA = max(128 * sA - 512 * tb, 0)
                loB = max(128 * sB - 512 * tb, 0)
                nA, nB = 512 - loA, 512 - loB
                # chunk A at psum cols [0:nA] (bank 0), chunk B at [512:512+nB]
                # (bank 1) so each start=True matmul owns a whole bank
                ps = ps_sc.tile([128, 1024], F32, tag="ps",
                                name=f"sc_{tb}_{h}_{j}")
                nc.tensor.matmul(
                    ps[:, 0:nA],
                    kT[po:po + 64, r * S + sA * 128: r * S + sA * 128 + 128],
                    qT[po:po + 64, r * S + 512 * tb + loA: r * S + 512 * (tb + 1)],
                    start=True, stop=True)
                nc.tensor.matmul(
                    ps[:, 512:512 + nB],
                    kT[po:po + 64, r * S + sB * 128: r * S + sB * 128 + 128],
                    qT[po:po + 64, r * S + 512 * tb + loB: r * S + 512 * (tb + 1)],
                    start=True, stop=True)
                if sA >= 4 * tb:
                    nc.vector.tensor_add(ps[:, 0:128], ps[:, 0:128], tri[:])
                if sB >= 4 * tb:
                    nc.vector.tensor_add(
                        ps[:, 512:512 + 128], ps[:, 512:512 + 128], tri[:])
                pr = pr_pool.tile([128, 1024], BF, tag="pr",
                                  name=f"pr_{tb}_{h}_{j}")
                if nA == 512:
                    # valid region is contiguous [0 : 512+nB] — one exp
                    nc.scalar.activation(
                        pr[:, 0:512 + nB], ps[:, 0:512 + nB], Exp)
                else:
                    nc.scalar.activation(pr[:, 0:nA], ps[:, 0:nA], Exp)
                    nc.scalar.activation(
                        pr[:, 512:512 + nB], ps[:, 512:512 + nB], Exp)
                return pr

            av_acc = {}

            def emit_av_pair(h, j, tb, pr):
                r, po = h // 2, 64 * (h % 2)
                sA, sB = 2 * j, 2 * j + 1
                loA = max(128 * sA - 512 * tb, 0)
                loB = max(128 * sB - 512 * tb, 0)
                nA, nB = 512 - loA, 512 - loB
                if j == 0:
                    av_acc[h] = ps_av.tile([128, 512], F32, tag="pa",
                                           name=f"av_{tb}_{h}")
                acc = av_acc[h]
                nc.tensor.matmul(
                    acc[0:65, loA:512],
                    v1[:, (h * 16 + sA) * 65:(h * 16 + sA) * 65 + 65],
                    pr[:, 0:nA],
                    start=(j == 0), stop=False)
                last = (j == 2 * tb + 1)
                nc.tensor.matmul(
                    acc[0:65, loB:512],
                    v1[:, (h * 16 + sB) * 65:(h * 16 + sB) * 65 + 65],
                    pr[:, 512:512 + nB],
                    start=False, stop=last)
                if last:
                    acc = av_acc.pop(h)
                    den = rc_pool.tile([1, 512], F32, tag="den")
                    nc.vector.tensor_copy(den[:], acc[64:65, :])
                    rc = rc_pool.tile([1, 512], F32, tag="rc")
                    nc.vector.reciprocal_approx_fast(rc[:], den[:])
                    rb = rb_pool.tile([64, 512], F32, tag="rb")
                    nc.gpsimd.partition_broadcast(rb[:], rc[:])
                    nc.vector.tensor_mul(
                        oT[po:po + 64, r * S + 512 * tb: r * S + 512 * tb + 512],
                        acc[0:64, :], rb[:])

            LA = 4

            def attention_block(tb, late_fills=None):
                batches = [(h, j) for h in range(HG) for j in range(2 * tb + 2)]
                nbatch = len(batches)
                late_start = 54
                fills_per = len(fill_q) / nbatch
                # spread late fills so ~3 chains remain to drain after the
                # final av pairs, covering the last normalize latency
                late_per = (len(late_fills) / (nbatch - late_start + 4)
                            if late_fills else 0.0)
                fill_credit = 0.0
                late_credit = 0.0
                pending = deque()
                for idx, (h, j) in enumerate(batches):
                    pr = emit_sc_pair(h, j, tb)
                    pending.append((h, j, pr))
                    if idx >= LA:
                        h0, j0, pr0 = pending.popleft()
                        emit_av_pair(h0, j0, tb, pr0)
                    fill_credit += fills_per
                    while fill_credit >= 1.0 and fill_q:
                        fill_q.popleft()()
                        fill_credit -= 1.0
                    if late_fills and idx >= late_start:
                        late_credit += late_per
                        while late_credit >= 1.0 and late_fills:
                            late_fills.popleft()()
                            late_credit -= 1.0
                while pending:
                    h0, j0, pr0 = pending.popleft()
                    emit_av_pair(h0, j0, tb, pr0)
                while fill_q:
                    fill_q.popleft()()
                if late_fills:
                    while late_fills:
                        late_fills.popleft()()

            # block 0 qkv inline, then attention blocks with fills
            for t in qkv_thunks(0):
                t()
            for tb in range(NB):
                if tb + 1 < NB:
                    fill_q.extend(qkv_thunks(tb + 1))
                if tb >= 1:
                    fill_q.extend(proj_thunks(tb - 1))
                late = None
                if tb == NB - 1:
                    late = deque(
                        (lambda m=m, h=h: projA_chain(m, h))
                        for m in range(12, 16) for h in range(2))
                attention_block(tb, late)
            for m in range(12, 16):
                for h in range(2):
                    projB_tail(m, h)
    nc.compile()
    return nc


def _make_masks():
    # tri[p, c] = 0 if c >= p else -1e10 (lower-triangle additive mask applied
    # to the first 128 valid columns of each diagonal key chunk)
    p = np.arange(128)[:, None]
    c = np.arange(128)[None, :]
    return np.where(c >= p, 0.0, -1e10).astype(np.float32)


def _make_in_maps(x, w_qkv, w_out):
    import ml_dtypes
    bf = ml_dtypes.bfloat16
    masks = _make_masks()
    scale = np.float32(DH ** -0.5)
    in_maps = []
    for c in range(NCORES):
        g = c % 2
        wq = w_qkv[:, g * 512:(g + 1) * 512] * scale
        wk = w_qkv[:, D + g * 512: D + (g + 1) * 512]
        wv = w_qkv[:, 2 * D + g * 512: 2 * D + (g + 1) * 512]
        w = np.concatenate([wq, wk, wv], axis=1)  # [D, 1536]
        # [128, ki*1536]: partition p, chunk ki -> w[ki*128 + p, :]
        w = w.reshape(KC, 128, 1536).transpose(1, 0, 2).reshape(128, -1)
        # xT [D, S] -> [128, ki*S]
        xt = np.ascontiguousarray(x[c // 2].T)  # [D, S]
        xt = xt.reshape(KC, 128, S).transpose(1, 0, 2).reshape(128, -1)
        # wout slice [512, D] -> [128, k*D]
        wo = w_out[g * 512:(g + 1) * 512, :]
        wo = wo.reshape(4, 128, D).transpose(1, 0, 2).reshape(128, -1)
        in_maps.append({
            "xt": np.ascontiguousarray(xt).astype(bf),
            "wqkv": np.ascontiguousarray(w).astype(bf),
            "wout": np.ascontiguousarray(wo).astype(bf),
            "masks": masks,
        })
    return in_maps


def kernel(x, w_qkv, w_out):
    from concourse.bass_utils import run_bass_kernel_spmd

    x = np.asarray(x, dtype=np.float32)
    w_qkv = np.asarray(w_qkv, dtype=np.float32)
    w_out = np.asarray(w_out, dtype=np.float32)
    assert x.shape == (B, S, D) and w_qkv.shape == (D, 3 * D) and w_out.shape == (D, D)

    if "nc" not in _CACHE:
        _CACHE["nc"] = _build_nc()
    nc = _CACHE["nc"]

    in_maps = _make_in_maps(x, w_qkv, w_out)
    res = run_bass_kernel_spmd(nc, in_maps, core_ids=list(range(NCORES)),
                               trace=False)
    out = np.empty((B, S, D), dtype=np.float32)
    for b in range(B):
        out[b] = (np.asarray(res.results[2 * b]["out"], dtype=np.float32)
                  + np.asarray(res.results[2 * b + 1]["out"], dtype=np.float32))
    return out
